# revision 4
# baseline (speedup 1.0000x reference)
"""Trainium2 Bass kernel for nn_BenchmarkFromHell (dense_cnn).

Strategy (8 NeuronCores, single SPMD launch):
  - Convs are batch-parallel: each core runs conv1+conv2 (+relu, square) for its
    32 samples via im2col matmuls using a flat-window trick (windows of the
    zero-padded 32x32 plane are contiguous slices, so im2col rows are plain
    strided DMAs; invalid output columns are discarded at eviction).
  - The pooling divisions and the 1e-12/1e-20 epsilons are dropped/folded:
    y is invariant to positive scaling of h (row-normalization), so only
    h^2 and n2 = sum(h^2) are needed.
  - lin->fc has no nonlinearity between, so each core folds its k-slice:
    Wfc_c = fc_w @ lin_w[:, k_slice]  (reads the 83.5MB shard once, f32r
    matmuls at full PE rate), then y_part = h2^2 @ Wfc_c^T with an appended
    ones-column producing n2 partials. k-slices are 13 psum-tile-aligned
    (98 real k-tiles padded to 104; core 7's shard is zero-padded).
  - h^2 features move between cores with a 1.7MB AllToAll.
  - Host combines: y = sum_c y_part / n2 + fc_b, then mean-abs normalize.
"""
import math
import numpy as np

# ---------------- constants (hardcoded problem shapes) ----------------
B = 256
NCORES = 8
BC = B // NCORES          # 32 samples per core
CH = 4                    # conv batch-chunk per core
NCHUNK = BC // CH         # 8
KT_TOT = 98               # 12544 / 128 k-tiles
KT_LOC = 13               # k-tiles per core (padded: 8*13 = 104)
KT_PAD = NCORES * KT_LOC  # 104
KCOL = KT_LOC * 128       # 1664 shard columns per core
PLANE = 1024              # 32*32 padded plane
HM = 132 + 2              # margins around the flat planes
XLEN = 2 + BC * PLANE + 132
H1LEN = 2 + CH * PLANE + 132

_S_C = sum(math.sin(c + 1) for c in range(5))
_S_D = sum(1.0 / (math.cos(d + 1e-9) + 1e-9) for d in range(5))
_S_E = sum(math.sqrt(e + 1) for e in range(5))
_S = _S_C * _S_D * _S_E
_TI = np.arange(23)
_TJ = (_TI * 7919) % 23
_TK = (_TJ * 1543) % 23


def _make_kernel_np(base, a, b, thrash, noise):
    base = np.asarray(base, np.float32)
    acc = base * np.float32(_S)
    acc = acc + (np.float32(a) * np.float32(b)) * acc.sum(
        axis=(2, 3), keepdims=True, dtype=np.float32)
    acc = acc + np.float32(np.asarray(thrash, np.float32)[_TI, _TJ, _TK].sum()
                           * 1e-12)
    r = np.asarray(noise, np.float32)
    for _ in range(3):
        r = r * (r + np.float32(1e-7))
    k = acc + r
    return k / (np.abs(k).mean(dtype=np.float32) + np.float32(1e-12))


# ---------------- bass program (built once per process) ----------------
_PROG = None      # (nc, in_names_order, runner)
_WCACHE = {}      # id(lin_w) -> per-core shard list


def _sap(base_ap, dims, offset):
    """Arbitrary strided AP view (overlapping windows etc.)."""
    import bass_rust
    a = base_ap.copy()
    a.ap = bass_rust.VecI64Pair([list(d) for d in dims])
    a.offset = int(offset)
    return a


def _build_program():
    import concourse.bass as bass
    import concourse.tile as tile
    from concourse import bacc, mybir
    from concourse.masks import make_identity
    from contextlib import ExitStack

    F32 = mybir.dt.float32
    F32R = mybir.dt.float32r
    ACT = mybir.ActivationFunctionType

    nc = bacc.Bacc(num_devices=NCORES)

    xflat = nc.dram_tensor("xflat", [XLEN], F32, kind="ExternalInput")
    w1rep = nc.dram_tensor("w1rep", [25, 128], F32, kind="ExternalInput")
    w2repA = nc.dram_tensor("w2repA", [128, 128], F32, kind="ExternalInput")
    w2repB = nc.dram_tensor("w2repB", [72, 128], F32, kind="ExternalInput")
    shard = nc.dram_tensor("shard", [12544, KCOL], F32R, kind="ExternalInput")
    fcwT = nc.dram_tensor("fcwT", [12544, 10], F32R, kind="ExternalInput")
    yout = nc.dram_tensor("yout", [2, 128, 11], F32, kind="ExternalOutput")

    with tile.TileContext(nc) as tc, ExitStack() as ctx:
        const = ctx.enter_context(tc.tile_pool(name="const", bufs=1))
        convp = ctx.enter_context(tc.tile_pool(name="convp", bufs=1))
        im1p = ctx.enter_context(tc.tile_pool(name="im1p", bufs=1))
        im2p = ctx.enter_context(tc.tile_pool(name="im2p", bufs=1))
        sqp = ctx.enter_context(tc.tile_pool(name="sqp", bufs=2))
        vp = ctx.enter_context(tc.tile_pool(name="vp", bufs=1))
        shp = ctx.enter_context(tc.tile_pool(name="shp", bufs=8))
        wfp = ctx.enter_context(tc.tile_pool(name="wfp", bufs=1))
        outp = ctx.enter_context(tc.tile_pool(name="outp", bufs=2))
        psA = ctx.enter_context(tc.tile_pool(name="psA", bufs=2, space="PSUM"))
        psB = ctx.enter_context(tc.tile_pool(name="psB", bufs=2, space="PSUM"))
        psW = ctx.enter_context(tc.tile_pool(name="psW", bufs=1, space="PSUM"))
        dram = ctx.enter_context(tc.tile_pool(name="dram", bufs=1, space="DRAM"))

        # ---- constants ----
        w1_sb = const.tile([25, 128], F32)
        nc.sync.dma_start(w1_sb[:], w1rep[:, :])
        w2A_sb = const.tile([128, 128], F32)
        nc.sync.dma_start(w2A_sb[:], w2repA[:, :])
        w2B_sb = const.tile([72, 128], F32)
        nc.sync.dma_start(w2B_sb[:], w2repB[:, :])
        fcw_sb = const.tile([128, KT_TOT, 10], F32R)
        nc.sync.dma_start(fcw_sb[:], fcwT.rearrange("(jt p) t -> p jt t", p=128))
        ident = const.tile([128, 128], F32)
        make_identity(nc, ident[:])

        # ---- persistent conv/feature buffers ----
        h1rep = convp.tile([128, H1LEN], F32)   # 16 replicas x 8 ch, padded planes
        nc.gpsimd.memset(h1rep[:], 0.0)
        v_loc = vp.tile([128, KT_PAD, BC], F32)  # (hw8*16+o, kt, b_local)
        nc.gpsimd.memset(v_loc[:], 0.0)

        h1ext = h1rep[:].ap[0][0]  # allocated free extent (elements)

        # ================= Wfc build (the big stream, f32r) =================
        # Wfc[t, k] = sum_j fc_w[t, j] * lin_w[j, k]; 4 col-groups of 416.
        wfcS = wfp.tile([10, KCOL], F32)
        wps = [psW.tile([10, 416], mybir.dt.float32, name=f"wps{g}") for g in range(4)]
        for jt in range(KT_TOT):
            st = shp.tile([128, KCOL], F32R, name="st", tag="st")
            nc.sync.dma_start(st[:], shard[jt * 128:(jt + 1) * 128, :])
            for g in range(4):
                nc.tensor.matmul(wps[g][:], fcw_sb[:, jt, :],
                                 st[:, g * 416:(g + 1) * 416],
                                 start=(jt == 0), stop=(jt == KT_TOT - 1),
                                 skip_group_check=True)
        for g in range(4):
            nc.scalar.activation(wfcS[:, g * 416:(g + 1) * 416], wps[g][:], ACT.Copy)

        # transpose Wfc -> wfcT_sb [128, 13, 11] (col 10 = ones for n2)
        wfcT_sb = wfp.tile([128, KT_LOC, 11], F32)
        nc.vector.memset(wfcT_sb[:, :, 10:11], 1.0)
        for t in range(KT_LOC):
            pst = psA.tile([128, 10], mybir.dt.float32, name="pst", tag="psA")
            nc.tensor.transpose(pst[:], wfcS[:, t * 128:(t + 1) * 128],
                                ident[0:10, 0:10])
            nc.scalar.activation(wfcT_sb[:, t, 0:10], pst[:], ACT.Copy)

        # ======================= convolutions =======================
        for c in range(NCHUNK):
            # ---- im2col1: 5 DMAs (one per dy); rows (dy*5+dx) ----
            im1 = im1p.tile([25, CH, 896], F32, name="im1", tag="im1")
            for dy in range(5):
                src = _sap(xflat[:], [[1, 5], [PLANE, CH], [1, 896]],
                           2 + (c * CH) * PLANE + dy * 32 - 2)
                nc.sync.dma_start(im1[dy * 5:(dy + 1) * 5, :, :], src)
            # ---- conv1 (K=25) + relu-evict into padded h1 planes ----
            for b in range(CH):
                for hh in range(2):
                    ps = psA.tile([128, 14, 32], mybir.dt.float32,
                                  name="c1ps", tag="psA")
                    nc.tensor.matmul(ps[:].rearrange("p a b -> p (a b)"),
                                     w1_sb[:], im1[:, b, hh * 448:(hh + 1) * 448],
                                     start=True, stop=True)
                    dst = _sap(h1rep[:], [[h1ext, 128], [32, 14], [1, 28]],
                               2 + b * PLANE + (2 + hh * 14) * 32 + 2)
                    nc.scalar.activation(dst, ps[:, :, 2:30], ACT.Relu)
            # ---- im2col2: 25 SBUF->SBUF DMAs from replicated h1 ----
            imA = im2p.tile([128, CH, 896], F32, name="imA", tag="imA")
            imB = im2p.tile([72, CH, 896], F32, name="imB", tag="imB")
            for od in range(25):
                dy, dx = od // 5, od % 5
                g = od % 16
                src = _sap(h1rep[:], [[h1ext, 8], [PLANE, CH], [1, 896]],
                           g * 8 * h1ext + 2 + dy * 32 + dx - 2)
                if od < 16:
                    nc.sync.dma_start(imA[od * 8:(od + 1) * 8, :, :], src)
                else:
                    nc.sync.dma_start(imB[(od - 16) * 8:(od - 15) * 8, :, :], src)
            # ---- conv2 (K=128+72) + relu + square into sqT [128, 784, CH] ----
            sqT = sqp.tile([128, 784, CH], F32, name="sqT", tag="sqT")
            for b in range(CH):
                for hh in range(2):
                    ps2 = psB.tile([128, 14, 32], mybir.dt.float32,
                                   name="c2ps", tag="psB")
                    nc.tensor.matmul(ps2[:].rearrange("p a b -> p (a b)"),
                                     w2A_sb[:], imA[:, b, hh * 448:(hh + 1) * 448],
                                     start=True, stop=False, skip_group_check=True)
                    nc.tensor.matmul(ps2[:].rearrange("p a b -> p (a b)"),
                                     w2B_sb[:], imB[:, b, hh * 448:(hh + 1) * 448],
                                     start=False, stop=True, skip_group_check=True)
                    sq_ext = sqT[:].ap[0][0]
                    dst = _sap(sqT[:], [[sq_ext, 128], [28 * CH, 14], [CH, 28]],
                               (hh * 14 * 28) * CH + b)
                    nc.scalar.activation(dst, ps2[:, :, 2:30], ACT.Relu)
            nc.vector.tensor_mul(sqT[:], sqT[:], sqT[:])
            # ---- v-scatter: 8 DMAs (one per hw8) into v_loc ----
            sq_ext = sqT[:].ap[0][0]
            vl_ext = v_loc[:].ap[0][0]
            for hw8 in range(8):
                src = _sap(sqT[:], [[sq_ext, 16], [8 * CH, 98], [1, CH]],
                           hw8 * 16 * sq_ext + hw8 * CH)
                dst = _sap(v_loc[:], [[vl_ext, 16], [BC, 98], [1, CH]],
                           hw8 * 16 * vl_ext + c * CH)
                nc.sync.dma_start(dst, src)

        # ======================= all-to-all =======================
        a2a_in = dram.tile([KT_PAD, 128, BC], F32)
        a2a_out = dram.tile([KT_PAD, 128, BC], F32)
        nc.sync.dma_start(a2a_in[:].rearrange("kt p b -> p kt b"), v_loc[:])
        nc.gpsimd.collective_compute(
            "AllToAll", mybir.AluOpType.bypass,
            replica_groups=[list(range(NCORES))],
            ins=[a2a_in[:].opt()], outs=[a2a_out[:].opt()])

        v_all = vp.tile([128, KT_LOC, B], F32)
        for s in range(NCORES):
            nc.sync.dma_start(
                v_all[:, :, s * BC:(s + 1) * BC],
                a2a_out[s * KT_LOC:(s + 1) * KT_LOC].rearrange("t p b -> p t b"))

        # =================== final tiny matmul ===================
        # y_part[b, t] (t<10) and n2_part (t=10): [2, 128, 11]
        for m in range(2):
            psy = psB.tile([128, 11], mybir.dt.float32, name="psy", tag="psB")
            for t in range(KT_LOC):
                nc.tensor.matmul(psy[:], v_all[:, t, m * 128:(m + 1) * 128],
                                 wfcT_sb[:, t, :],
                                 start=(t == 0), stop=(t == KT_LOC - 1),
                                 skip_group_check=True)
            ysb = outp.tile([128, 11], F32, name="ysb", tag="ysb")
            nc.scalar.activation(ysb[:], psy[:], ACT.Copy)
            nc.sync.dma_start(yout[m, :, :], ysb[:])

    nc.finalize()
    return nc


def _get_prog():
    global _PROG
    if _PROG is None:
        _PROG = _build_program()
    return _PROG


def _prep_inputs(x, base1, a1, b1, thrash1, noise1, base2, a2, b2, thrash2,
                 noise2, lin_w, fc_w):
    """Host-side weight prep -> per-core input maps."""
    w1 = _make_kernel_np(base1, a1, b1, thrash1, noise1)   # [8,1,5,5]
    w2 = _make_kernel_np(base2, a2, b2, thrash2, noise2)   # [16,8,5,5]

    # w1rep [25, 128]: rows (dy*5+dx), cols (g*8 + o), 16 replicas
    w1col = w1[:, 0, :, :].transpose(1, 2, 0).reshape(25, 8)
    w1rep = np.ascontiguousarray(np.tile(w1col, (1, 16)), np.float32)
    # w2rep [200, 128]: rows (od*8 + i), cols (r*16 + o), 8 replicas
    w2col = w2.transpose(2, 3, 1, 0).reshape(200, 16)
    w2rep = np.ascontiguousarray(np.tile(w2col, (1, 8)), np.float32)

    # permuted lin_w columns: k = hw0*128 + hw8*16 + o  <->  o*784 + hw0*8 + hw8
    key = (id(lin_w), lin_w.shape)
    shards = _WCACHE.get(key)
    if shards is None:
        lw = np.asarray(lin_w, np.float32)
        perm = np.ascontiguousarray(
            lw.reshape(12544, 16, 98, 8).transpose(0, 2, 3, 1)).reshape(12544, 12544)
        shards = []
        for c in range(NCORES):
            lo = c * KCOL
            hi = min(lo + KCOL, 12544)
            if hi - lo == KCOL:
                shards.append(np.ascontiguousarray(perm[:, lo:hi]))
            else:
                s = np.zeros((12544, KCOL), np.float32)
                s[:, :hi - lo] = perm[:, lo:hi]
                shards.append(s)
        del perm
        _WCACHE.clear()
        _WCACHE[key] = shards

    fcwT_np = np.ascontiguousarray(np.asarray(fc_w, np.float32).T)  # [12544,10]

    xf = np.asarray(x, np.float32).reshape(B, 28, 28)
    in_maps = []
    for c in range(NCORES):
        xbuf = np.zeros(XLEN, np.float32)
        view = xbuf[2:2 + BC * PLANE].reshape(BC, 32, 32)
        view[:, 2:30, 2:30] = xf[c * BC:(c + 1) * BC]
        in_maps.append({
            "xflat": xbuf,
            "w1rep": w1rep,
            "w2repA": w2rep[:128],
            "w2repB": np.ascontiguousarray(w2rep[128:]),
            "shard": shards[c],
            "fcwT": fcwT_np,
        })
    return in_maps


def _combine(results, fc_b):
    """Host combine: sum partials across cores, normalize."""
    acc = np.zeros((2, 128, 11), np.float64)
    for r in results:
        acc += r["yout"].astype(np.float64)
    flat = acc.reshape(256, 11)
    y_raw = flat[:, :10]
    n2 = flat[:, 10:11]
    denom = (np.sqrt(n2) + 1e-20) ** 2
    y = y_raw / denom + np.asarray(fc_b, np.float64)[None, :]
    y = y / (np.abs(y).mean() + 1e-30)
    return y.astype(np.float32)


_RUNNER = None


def _get_runner(nc):
    """Cached jitted SPMD executor (mirrors bass2jax.run_bass_via_pjrt)."""
    global _RUNNER
    if _RUNNER is not None:
        return _RUNNER
    import jax
    import numpy as _np
    from jax.sharding import Mesh, PartitionSpec
    from jax.experimental.shard_map import shard_map
    from concourse import mybir
    from concourse.bass2jax import _bass_exec_p, install_neuronx_cc_hook

    install_neuronx_cc_hook()
    partition_name = (nc.partition_id_tensor.name
                      if nc.partition_id_tensor else None)
    in_names, out_names, out_avals, zero_shapes = [], [], [], []
    for alloc in nc.m.functions[0].allocations:
        if not isinstance(alloc, mybir.MemoryLocationSet):
            continue
        name = alloc.memorylocations[0].name
        if alloc.kind == "ExternalInput":
            if name != partition_name:
                in_names.append(name)
        elif alloc.kind == "ExternalOutput":
            shape = tuple(alloc.tensor_shape)
            dtype = mybir.dt.np(alloc.dtype)
            out_names.append(name)
            out_avals.append(jax.core.ShapedArray(shape, dtype))
            zero_shapes.append((shape, dtype))
    n_params = len(in_names)
    n_outs = len(out_names)
    all_in_names = in_names + out_names + ([partition_name] if partition_name else [])
    donate = tuple(range(n_params, n_params + n_outs))

    from concourse.bass2jax import partition_id_tensor

    def _body(*args):
        operands = list(args)
        if partition_name is not None:
            operands.append(partition_id_tensor())
        outs = _bass_exec_p.bind(
            *operands, out_avals=tuple(out_avals), in_names=tuple(all_in_names),
            out_names=tuple(out_names), lowering_input_output_aliases=(),
            sim_require_finite=True, sim_require_nnan=True, nc=nc)
        return tuple(outs)

    devices = jax.devices()[:NCORES]
    mesh = Mesh(_np.asarray(devices), ("core",))
    in_specs = (PartitionSpec("core"),) * (n_params + n_outs)
    out_specs = (PartitionSpec("core"),) * n_outs
    sharded = jax.jit(shard_map(_body, mesh=mesh, in_specs=in_specs,
                                out_specs=out_specs, check_rep=False),
                      donate_argnums=donate, keep_unused=True)

    def run(in_maps):
        concat_in = [np.concatenate([np.asarray(m[name]) for m in in_maps], axis=0)
                     if np.asarray(in_maps[0][name]).ndim > 0 else
                     np.stack([np.asarray(m[name]) for m in in_maps])
                     for name in in_names]
        # 1-D inputs: concatenate along axis 0 works the same
        concat_zeros = [np.zeros((NCORES * s[0], *s[1:]), d)
                        for (s, d) in zero_shapes]
        out_arrs = sharded(*concat_in, *concat_zeros)
        return [{name: np.asarray(out_arrs[i]).reshape(NCORES, *out_avals[i].shape)[c]
                 for i, name in enumerate(out_names)}
                for c in range(NCORES)]

    _RUNNER = run
    return run


def kernel(x, base1, a1, b1, thrash1, noise1, base2, a2, b2, thrash2, noise2,
           lin_w, fc_w, fc_b):
    in_maps = _prep_inputs(x, base1, a1, b1, thrash1, noise1,
                           base2, a2, b2, thrash2, noise2, lin_w, fc_w)
    nc = _get_prog()
    run = _get_runner(nc)
    results = run(in_maps)
    return _combine(results, fc_b)


def run_traced(inputs):
    """Run via run_bass_kernel_spmd(trace=True); returns (y, exec_time_ns)."""
    from concourse.bass_utils import run_bass_kernel_spmd
    kw = {k: v for k, v in inputs.items() if k != "fc_b"}
    in_maps = _prep_inputs(**kw)
    nc = _get_prog()
    res = run_bass_kernel_spmd(nc, in_maps, core_ids=list(range(NCORES)),
                               trace=True)
    y = _combine(res.results, inputs["fc_b"])
    return y, res.exec_time_ns, res


# revision 6
# speedup vs baseline: 1.3319x; 1.3319x over previous
"""Trainium2 Bass kernel for nn_BenchmarkFromHell (dense_cnn).

Strategy (8 NeuronCores, single SPMD launch):
  - Convs are batch-parallel: each core runs conv1+conv2 (+relu, square) for its
    32 samples via im2col matmuls using a flat-window trick (windows of the
    zero-padded 32x32 plane are contiguous slices, so im2col rows are plain
    strided DMAs; invalid output columns are discarded at eviction).
  - The pooling divisions and the 1e-12/1e-20 epsilons are dropped/folded:
    y is invariant to positive scaling of h (row-normalization), so only
    h^2 and n2 = sum(h^2) are needed.
  - lin->fc has no nonlinearity between, so each core folds its k-slice:
    Wfc_c = fc_w @ lin_w[:, k_slice]  (reads the 83.5MB shard once, f32r
    matmuls at full PE rate), then y_part = h2^2 @ Wfc_c^T with an appended
    ones-column producing n2 partials. k-slices are 13 psum-tile-aligned
    (98 real k-tiles padded to 104; core 7's shard is zero-padded).
  - h^2 features move between cores with a 1.7MB AllToAll.
  - Host combines: y = sum_c y_part / n2 + fc_b, then mean-abs normalize.
"""
import math
import numpy as np

# ---------------- constants (hardcoded problem shapes) ----------------
B = 256
NCORES = 8
BC = B // NCORES          # 32 samples per core
CH = 4                    # conv batch-chunk per core
NCHUNK = BC // CH         # 8
KT_TOT = 98               # 12544 / 128 k-tiles
KT_LOC = 13               # k-tiles per core (padded: 8*13 = 104)
KT_PAD = NCORES * KT_LOC  # 104
KCOL = KT_LOC * 128       # 1664 shard columns per core
PLANE = 1024              # 32*32 padded plane
HM = 132 + 2              # margins around the flat planes
XLEN = 2 + BC * PLANE + 132
H1LEN = 2 + CH * PLANE + 132

_S_C = sum(math.sin(c + 1) for c in range(5))
_S_D = sum(1.0 / (math.cos(d + 1e-9) + 1e-9) for d in range(5))
_S_E = sum(math.sqrt(e + 1) for e in range(5))
_S = _S_C * _S_D * _S_E
_TI = np.arange(23)
_TJ = (_TI * 7919) % 23
_TK = (_TJ * 1543) % 23


def _make_kernel_np(base, a, b, thrash, noise):
    base = np.asarray(base, np.float32)
    acc = base * np.float32(_S)
    acc = acc + (np.float32(a) * np.float32(b)) * acc.sum(
        axis=(2, 3), keepdims=True, dtype=np.float32)
    acc = acc + np.float32(np.asarray(thrash, np.float32)[_TI, _TJ, _TK].sum()
                           * 1e-12)
    r = np.asarray(noise, np.float32)
    for _ in range(3):
        r = r * (r + np.float32(1e-7))
    k = acc + r
    return k / (np.abs(k).mean(dtype=np.float32) + np.float32(1e-12))


# ---------------- bass program (built once per process) ----------------
_PROG = None      # (nc, in_names_order, runner)
_WCACHE = {}      # id(lin_w) -> per-core shard list


def _sap(base_ap, dims, offset):
    """Arbitrary strided AP view (overlapping windows etc.)."""
    import bass_rust
    a = base_ap.copy()
    a.ap = bass_rust.VecI64Pair([list(d) for d in dims])
    a.offset = int(offset)
    return a


def _build_program():
    import concourse.bass as bass
    import concourse.tile as tile
    from concourse import bacc, mybir
    from concourse.masks import make_identity
    from contextlib import ExitStack

    F32 = mybir.dt.float32
    F32R = mybir.dt.float32r
    ACT = mybir.ActivationFunctionType

    nc = bacc.Bacc(num_devices=NCORES)

    xflat = nc.dram_tensor("xflat", [XLEN], F32R, kind="ExternalInput")
    w1rep = nc.dram_tensor("w1rep", [25, 128], F32R, kind="ExternalInput")
    w2repA = nc.dram_tensor("w2repA", [128, 128], F32R, kind="ExternalInput")
    w2repB = nc.dram_tensor("w2repB", [72, 128], F32R, kind="ExternalInput")
    shard = nc.dram_tensor("shard", [12544, KCOL], F32R, kind="ExternalInput")
    fcwT = nc.dram_tensor("fcwT", [12544, 10], F32R, kind="ExternalInput")
    yout = nc.dram_tensor("yout", [2, 128, 12], F32, kind="ExternalOutput")

    with tile.TileContext(nc) as tc, ExitStack() as ctx:
        const = ctx.enter_context(tc.tile_pool(name="const", bufs=1))
        convp = ctx.enter_context(tc.tile_pool(name="convp", bufs=1))
        im1p = ctx.enter_context(tc.tile_pool(name="im1p", bufs=2))
        im2p = ctx.enter_context(tc.tile_pool(name="im2p", bufs=2))
        sqp = ctx.enter_context(tc.tile_pool(name="sqp", bufs=2))
        vp = ctx.enter_context(tc.tile_pool(name="vp", bufs=1))
        shp = ctx.enter_context(tc.tile_pool(name="shp", bufs=6))
        wfp = ctx.enter_context(tc.tile_pool(name="wfp", bufs=1))
        outp = ctx.enter_context(tc.tile_pool(name="outp", bufs=2))
        psA = ctx.enter_context(tc.tile_pool(name="psA", bufs=2, space="PSUM"))
        psB = ctx.enter_context(tc.tile_pool(name="psB", bufs=2, space="PSUM"))
        psW = ctx.enter_context(tc.tile_pool(name="psW", bufs=1, space="PSUM"))
        dram = ctx.enter_context(tc.tile_pool(name="dram", bufs=1, space="DRAM"))

        # ---- constants ----
        w1_sb = const.tile([25, 128], F32R)
        nc.sync.dma_start(w1_sb[:], w1rep[:, :])
        w2A_sb = const.tile([128, 128], F32R)
        nc.sync.dma_start(w2A_sb[:], w2repA[:, :])
        w2B_sb = const.tile([72, 128], F32R)
        nc.sync.dma_start(w2B_sb[:], w2repB[:, :])
        fcw_sb = const.tile([128, KT_TOT, 10], F32R)
        nc.sync.dma_start(fcw_sb[:], fcwT.rearrange("(jt p) t -> p jt t", p=128))
        ident = const.tile([128, 128], F32)
        make_identity(nc, ident[:])

        # ---- persistent conv/feature buffers ----
        h1rep = convp.tile([128, H1LEN], F32R)  # 16 replicas x 8 ch, padded planes
        nc.gpsimd.memset(h1rep[:].bitcast(F32), 0.0)
        v_loc = vp.tile([128, KT_PAD, BC], F32R)  # (hw8*16+o, kt, b_local)
        nc.gpsimd.memset(v_loc[:].bitcast(F32), 0.0)

        h1ext = h1rep[:].ap[0][0]  # allocated free extent (elements)

        def conv_chunk(c):
            # ---- im2col1: 5 DMAs (one per dy); rows (dy*5+dx) ----
            im1 = im1p.tile([25, CH, 896], F32R, name="im1", tag="im1")
            for dy in range(5):
                src = _sap(xflat[:], [[1, 5], [PLANE, CH], [1, 896]],
                           2 + (c * CH) * PLANE + dy * 32 - 2)
                nc.gpsimd.dma_start(im1[dy * 5:(dy + 1) * 5, :, :], src)
            # ---- conv1 (K=25) + relu-evict into padded h1 planes ----
            for b in range(CH):
                for hh in range(2):
                    ps = psA.tile([128, 14, 32], mybir.dt.float32,
                                  name="c1ps", tag="psA")
                    nc.tensor.matmul(ps[:].rearrange("p a b -> p (a b)"),
                                     w1_sb[:], im1[:, b, hh * 448:(hh + 1) * 448],
                                     start=True, stop=True)
                    dst = _sap(h1rep[:], [[h1ext, 128], [32, 14], [1, 28]],
                               2 + b * PLANE + (2 + hh * 14) * 32 + 2)
                    nc.scalar.activation(dst, ps[:, :, 2:30], ACT.Relu)
            # ---- im2col2: 25 SBUF->SBUF DMAs from replicated h1 ----
            imA = im2p.tile([128, CH, 896], F32R, name="imA", tag="imA")
            imB = im2p.tile([72, CH, 896], F32R, name="imB", tag="imB")
            for od in range(25):
                dy, dx = od // 5, od % 5
                g = od % 16
                src = _sap(h1rep[:], [[h1ext, 8], [PLANE, CH], [1, 896]],
                           g * 8 * h1ext + 2 + dy * 32 + dx - 2)
                dst = (imA[od * 8:(od + 1) * 8, :, :] if od < 16
                       else imB[(od - 16) * 8:(od - 15) * 8, :, :])
                if od % 2 == 0:
                    nc.sync.dma_start(dst, src)
                else:
                    nc.scalar.dma_start(dst, src)
            # ---- conv2 (K=128+72) + relu into sqT [128, 784, CH] ----
            sqT = sqp.tile([128, 784, CH], F32R, name="sqT", tag="sqT")
            for b in range(CH):
                for hh in range(2):
                    ps2 = psB.tile([128, 14, 32], mybir.dt.float32,
                                   name="c2ps", tag="psB")
                    nc.tensor.matmul(ps2[:].rearrange("p a b -> p (a b)"),
                                     w2A_sb[:], imA[:, b, hh * 448:(hh + 1) * 448],
                                     start=True, stop=False, skip_group_check=True)
                    nc.tensor.matmul(ps2[:].rearrange("p a b -> p (a b)"),
                                     w2B_sb[:], imB[:, b, hh * 448:(hh + 1) * 448],
                                     start=False, stop=True, skip_group_check=True)
                    sq_ext = sqT[:].ap[0][0]
                    dst = _sap(sqT[:], [[sq_ext, 128], [28 * CH, 14], [CH, 28]],
                               (hh * 14 * 28) * CH + b)
                    nc.scalar.activation(dst, ps2[:, :, 2:30], ACT.Relu)
            nc.vector.tensor_mul(sqT[:], sqT[:], sqT[:])
            # ---- v-scatter: 8 DMAs (one per hw8) into v_loc ----
            sq_ext = sqT[:].ap[0][0]
            vl_ext = v_loc[:].ap[0][0]
            for hw8 in range(8):
                src = _sap(sqT[:], [[sq_ext, 16], [8 * CH, 98], [1, CH]],
                           hw8 * 16 * sq_ext + hw8 * CH)
                dst = _sap(v_loc[:], [[vl_ext, 16], [BC, 98], [1, CH]],
                           hw8 * 16 * vl_ext + c * CH)
                nc.gpsimd.dma_start(dst, src)

        # ============ interleaved: Wfc stream + conv chunks ============
        CHUNK_AT = {4 + 12 * i: i for i in range(NCHUNK)}
        wfcS = wfp.tile([10, KCOL], F32)
        wps = [psW.tile([10, 416], mybir.dt.float32, name=f"wps{g}", tag=f"wps{g}")
               for g in range(4)]
        for jt in range(KT_TOT):
            st = shp.tile([128, KCOL], F32R, name="st", tag="st")
            nc.sync.dma_start(st[:], shard[jt * 128:(jt + 1) * 128, :])
            for g in range(4):
                nc.tensor.matmul(wps[g][:], fcw_sb[:, jt, :],
                                 st[:, g * 416:(g + 1) * 416],
                                 start=(jt == 0), stop=(jt == KT_TOT - 1),
                                 skip_group_check=True)
            if jt in CHUNK_AT:
                conv_chunk(CHUNK_AT[jt])
        for g in range(4):
            nc.scalar.activation(wfcS[:, g * 416:(g + 1) * 416], wps[g][:], ACT.Copy)

        # transpose Wfc -> wfcT_sb [128, 13, 12] (col 10 = ones; col 11 = 0)
        wfcT_sb = wfp.tile([128, KT_LOC, 12], F32R)
        nc.vector.memset(wfcT_sb[:, :, 10:11].bitcast(F32), 1.0)
        nc.vector.memset(wfcT_sb[:, :, 11:12].bitcast(F32), 0.0)
        for t in range(KT_LOC):
            pst = psA.tile([128, 10], mybir.dt.float32, name="pst", tag="psA")
            nc.tensor.transpose(pst[:], wfcS[:, t * 128:(t + 1) * 128],
                                ident[0:10, 0:10])
            nc.scalar.activation(wfcT_sb[:, t, 0:10], pst[:], ACT.Copy)

        # ======================= all-to-all =======================
        a2a_in = dram.tile([KT_PAD, 128, BC], F32R)
        a2a_out = dram.tile([KT_PAD, 128, BC], F32R)
        nc.sync.dma_start(a2a_in[:].rearrange("kt p b -> p kt b"), v_loc[:])
        nc.gpsimd.collective_compute(
            "AllToAll", mybir.AluOpType.bypass,
            replica_groups=[list(range(NCORES))],
            ins=[a2a_in[:].opt()], outs=[a2a_out[:].opt()])

        v_all = vp.tile([128, KT_LOC, B], F32R)
        for s in range(NCORES):
            nc.sync.dma_start(
                v_all[:, :, s * BC:(s + 1) * BC],
                a2a_out[s * KT_LOC:(s + 1) * KT_LOC].rearrange("t p b -> p t b"))

        # =================== final tiny matmul ===================
        for m in range(2):
            psy = psB.tile([128, 12], mybir.dt.float32, name="psy", tag="psB")
            for t in range(KT_LOC):
                nc.tensor.matmul(psy[:], v_all[:, t, m * 128:(m + 1) * 128],
                                 wfcT_sb[:, t, :],
                                 start=(t == 0), stop=(t == KT_LOC - 1),
                                 skip_group_check=True)
            ysb = outp.tile([128, 12], F32, name="ysb", tag="ysb")
            nc.scalar.activation(ysb[:], psy[:], ACT.Copy)
            nc.sync.dma_start(yout[m, :, :], ysb[:])

    nc.finalize()
    return nc


def _get_prog():
    global _PROG
    if _PROG is None:
        _PROG = _build_program()
    return _PROG


def _prep_inputs(x, base1, a1, b1, thrash1, noise1, base2, a2, b2, thrash2,
                 noise2, lin_w, fc_w):
    """Host-side weight prep -> per-core input maps."""
    w1 = _make_kernel_np(base1, a1, b1, thrash1, noise1)   # [8,1,5,5]
    w2 = _make_kernel_np(base2, a2, b2, thrash2, noise2)   # [16,8,5,5]

    # w1rep [25, 128]: rows (dy*5+dx), cols (g*8 + o), 16 replicas
    w1col = w1[:, 0, :, :].transpose(1, 2, 0).reshape(25, 8)
    w1rep = np.ascontiguousarray(np.tile(w1col, (1, 16)), np.float32)
    # w2rep [200, 128]: rows (od*8 + i), cols (r*16 + o), 8 replicas
    w2col = w2.transpose(2, 3, 1, 0).reshape(200, 16)
    w2rep = np.ascontiguousarray(np.tile(w2col, (1, 8)), np.float32)

    # permuted lin_w columns: k = hw0*128 + hw8*16 + o  <->  o*784 + hw0*8 + hw8
    key = (id(lin_w), lin_w.shape)
    shards = _WCACHE.get(key)
    if shards is None:
        lw = np.asarray(lin_w, np.float32)
        perm = np.ascontiguousarray(
            lw.reshape(12544, 16, 98, 8).transpose(0, 2, 3, 1)).reshape(12544, 12544)
        shards = []
        for c in range(NCORES):
            lo = c * KCOL
            hi = min(lo + KCOL, 12544)
            if hi - lo == KCOL:
                shards.append(np.ascontiguousarray(perm[:, lo:hi]))
            else:
                s = np.zeros((12544, KCOL), np.float32)
                s[:, :hi - lo] = perm[:, lo:hi]
                shards.append(s)
        del perm
        _WCACHE.clear()
        _WCACHE[key] = shards

    fcwT_np = np.ascontiguousarray(np.asarray(fc_w, np.float32).T)  # [12544,10]

    xf = np.asarray(x, np.float32).reshape(B, 28, 28)
    in_maps = []
    for c in range(NCORES):
        xbuf = np.zeros(XLEN, np.float32)
        view = xbuf[2:2 + BC * PLANE].reshape(BC, 32, 32)
        view[:, 2:30, 2:30] = xf[c * BC:(c + 1) * BC]
        in_maps.append({
            "xflat": xbuf,
            "w1rep": w1rep,
            "w2repA": w2rep[:128],
            "w2repB": np.ascontiguousarray(w2rep[128:]),
            "shard": shards[c],
            "fcwT": fcwT_np,
        })
    return in_maps


def _combine(results, fc_b):
    """Host combine: sum partials across cores, normalize."""
    acc = np.zeros((2, 128, 12), np.float64)
    for r in results:
        acc += r["yout"].astype(np.float64)
    flat = acc.reshape(256, 12)
    y_raw = flat[:, :10]
    n2 = flat[:, 10:11]
    denom = (np.sqrt(n2) + 1e-20) ** 2
    y = y_raw / denom + np.asarray(fc_b, np.float64)[None, :]
    y = y / (np.abs(y).mean() + 1e-30)
    return y.astype(np.float32)


_RUNNER = None


def _get_runner(nc):
    """Cached jitted SPMD executor (mirrors bass2jax.run_bass_via_pjrt)."""
    global _RUNNER
    if _RUNNER is not None:
        return _RUNNER
    import jax
    import numpy as _np
    from jax.sharding import Mesh, PartitionSpec
    from jax.experimental.shard_map import shard_map
    from concourse import mybir
    from concourse.bass2jax import _bass_exec_p, install_neuronx_cc_hook

    install_neuronx_cc_hook()
    partition_name = (nc.partition_id_tensor.name
                      if nc.partition_id_tensor else None)
    in_names, out_names, out_avals, zero_shapes = [], [], [], []
    for alloc in nc.m.functions[0].allocations:
        if not isinstance(alloc, mybir.MemoryLocationSet):
            continue
        name = alloc.memorylocations[0].name
        if alloc.kind == "ExternalInput":
            if name != partition_name:
                in_names.append(name)
        elif alloc.kind == "ExternalOutput":
            shape = tuple(alloc.tensor_shape)
            dtype = mybir.dt.np(alloc.dtype)
            out_names.append(name)
            out_avals.append(jax.core.ShapedArray(shape, dtype))
            zero_shapes.append((shape, dtype))
    n_params = len(in_names)
    n_outs = len(out_names)
    all_in_names = in_names + out_names + ([partition_name] if partition_name else [])
    donate = tuple(range(n_params, n_params + n_outs))

    from concourse.bass2jax import partition_id_tensor

    def _body(*args):
        operands = list(args)
        if partition_name is not None:
            operands.append(partition_id_tensor())
        outs = _bass_exec_p.bind(
            *operands, out_avals=tuple(out_avals), in_names=tuple(all_in_names),
            out_names=tuple(out_names), lowering_input_output_aliases=(),
            sim_require_finite=True, sim_require_nnan=True, nc=nc)
        return tuple(outs)

    devices = jax.devices()[:NCORES]
    mesh = Mesh(_np.asarray(devices), ("core",))
    in_specs = (PartitionSpec("core"),) * (n_params + n_outs)
    out_specs = (PartitionSpec("core"),) * n_outs
    sharded = jax.jit(shard_map(_body, mesh=mesh, in_specs=in_specs,
                                out_specs=out_specs, check_rep=False),
                      donate_argnums=donate, keep_unused=True)

    def run(in_maps):
        concat_in = [np.concatenate([np.asarray(m[name]) for m in in_maps], axis=0)
                     if np.asarray(in_maps[0][name]).ndim > 0 else
                     np.stack([np.asarray(m[name]) for m in in_maps])
                     for name in in_names]
        # 1-D inputs: concatenate along axis 0 works the same
        concat_zeros = [np.zeros((NCORES * s[0], *s[1:]), d)
                        for (s, d) in zero_shapes]
        out_arrs = sharded(*concat_in, *concat_zeros)
        return [{name: np.asarray(out_arrs[i]).reshape(NCORES, *out_avals[i].shape)[c]
                 for i, name in enumerate(out_names)}
                for c in range(NCORES)]

    _RUNNER = run
    return run


def kernel(x, base1, a1, b1, thrash1, noise1, base2, a2, b2, thrash2, noise2,
           lin_w, fc_w, fc_b):
    in_maps = _prep_inputs(x, base1, a1, b1, thrash1, noise1,
                           base2, a2, b2, thrash2, noise2, lin_w, fc_w)
    nc = _get_prog()
    run = _get_runner(nc)
    results = run(in_maps)
    return _combine(results, fc_b)


def run_traced(inputs):
    """Run via run_bass_kernel_spmd(trace=True); returns (y, exec_time_ns)."""
    from concourse.bass_utils import run_bass_kernel_spmd
    kw = {k: v for k, v in inputs.items() if k != "fc_b"}
    in_maps = _prep_inputs(**kw)
    nc = _get_prog()
    res = run_bass_kernel_spmd(nc, in_maps, core_ids=list(range(NCORES)),
                               trace=True)
    y = _combine(res.results, inputs["fc_b"])
    return y, res.exec_time_ns, res
